# revision 51
# baseline (speedup 1.0000x reference)
"""Single-head attention (B=4, S=4096, E=1024, D=128) on 8 TRN2 NeuronCores.

Sharding: 8 cores = (batch b in 0..3) x (query-half h in 0..1). Each core
computes the attention output for its 2048 queries over the full 4096-key
sequence of its batch. K/V projections are recomputed per core (cheaper than
cross-core exchange). Inputs are pre-transposed on host to xT [E, S] and the
sequence axis is rotated so each core's query rows are columns 0..2047 of its
own xT (softmax is permutation-invariant over keys, so the rotation is free).

Per-core kernel (bf16 matmul operands, fp32 PSUM accumulation; a float32r
variant is kept as a higher-precision fallback):
  phase 1: stream xT k-blocks (fused multi-chunk DMAs on the sync HWDGE
           queue), compute KT[D,S] and VT via PE matmuls (contraction over E
           in 8 chunks of 128), PE-transpose VT into V[k,D] tiles, and
           QT[D,2048]. 1/sqrt(D) is folded into Wq on the host.
  phase 2: per q-block (512 queries) x k-chunk pair (2x128 keys):
           scoresT[k,q] = KTc.T @ QTblk        (PE -> PSUM, N=512)
           expT = exp(scoresT)                 (one ACT pass per 2 chunks)
           oT  += Vc.T @ expT                  (PE, PSUM accum over k)
           denominator: chunk pairs are pair-summed on DVE (2 levels), then
           one ones-matmul per 4 chunks accumulates column sums; the
           ones-matmul is emitted 2 iterations late so the DVE adds stay off
           the in-order PE stream's critical path. Scores for iteration i+1
           are emitted before iteration i's AV so the PE covers exp latency.
           Finally oT * reciprocal_approx_fast(den) on DVE, DMA out.
  No max-subtraction: scores are bounded (|s| <~ 5) so plain exp is safe,
  which removes all flash-attention online-softmax rescaling and lets the
  denominator be accumulated linearly.
"""

import math
import sys
from contextlib import ExitStack

import numpy as np

if "/opt/trn_rl_repo" not in sys.path:
    sys.path.insert(0, "/opt/trn_rl_repo")

import concourse.bass as bass  # noqa: E402
import concourse.tile as tile  # noqa: E402
from concourse import bacc, mybir  # noqa: E402
from concourse.bass_utils import run_bass_kernel_spmd  # noqa: E402
from concourse.masks import make_identity  # noqa: E402

F32 = mybir.dt.float32
F32R = mybir.dt.float32r
BF16 = mybir.dt.bfloat16

B, S, E, D = 4, 4096, 1024, 128
N_CORES = 8
QH = S // 2  # queries per core


def _r(ap):
    return ap.bitcast(F32R)


def round_fp32r(a):
    """Round-to-nearest-even fp32 -> fp32r (1s/8e/11m, low 12 bits zero)."""
    a = np.ascontiguousarray(a, dtype=np.float32)
    u = a.view(np.uint32)
    low = u & np.uint32(0xFFF)
    lsb = (u >> np.uint32(12)) & np.uint32(1)
    round_up = (low > 0x800) | ((low == 0x800) & (lsb == 1))
    u2 = (u & np.uint32(0xFFFFF000)) + (round_up.astype(np.uint32) << np.uint32(12))
    return u2.view(np.float32)


def build_nc(S_kv, S_q, E_, D_, KB=512, QB=512, mm_dt=F32R, fused=False):
    """Build the per-core Bass program."""
    EC = E_ // 128  # E chunks (contraction)
    NKB = S_kv // KB  # projection k-blocks
    NQB = S_q // QB  # attention q-blocks
    NKC = S_kv // 128  # attention k-chunks
    TPB = KB // 128  # transposes per k-block
    NKP = NKC // 2  # attention k-chunk pairs

    nc = bacc.Bacc(
        "TRN2",
        target_bir_lowering=False,
        debug=False,
        enable_asserts=False,
        num_devices=1,
    )
    xT = nc.dram_tensor("xT", [E_, S_kv], mm_dt, kind="ExternalInput")
    wq = nc.dram_tensor("wq", [E_, D_], mm_dt, kind="ExternalInput")
    wk = nc.dram_tensor("wk", [E_, D_], mm_dt, kind="ExternalInput")
    wv = nc.dram_tensor("wv", [E_, D_], mm_dt, kind="ExternalInput")
    oT = nc.dram_tensor("oT", [D_, S_q], F32, kind="ExternalOutput")

    with tile.TileContext(nc) as tc, ExitStack() as ctx:
        consts = ctx.enter_context(tc.tile_pool(name="consts", bufs=1))
        persist = ctx.enter_context(tc.tile_pool(name="persist", bufs=1))
        xpool = ctx.enter_context(tc.tile_pool(name="xblk", bufs=3))
        vt_pool = ctx.enter_context(tc.tile_pool(name="vt", bufs=2))
        p_pool = ctx.enter_context(tc.tile_pool(name="pchunk", bufs=8))
        pair_pool = ctx.enter_context(tc.tile_pool(name="pairs", bufs=14))
        o_pool = ctx.enter_context(tc.tile_pool(name="osb", bufs=4))

        # wk gates the very first matmul: scalar HWDGE queue (idle at start);
        # wv/wq/consts on gpsimd; xT stream on sync.
        w_sb = {}
        for name, w in (("wk", wk), ("wv", wv), ("wq", wq)):
            t = consts.tile([128, EC, D_], mm_dt, tag=f"w_{name}")
            eng = nc.scalar if name == "wk" else nc.gpsimd
            eng.dma_start(out=t, in_=w.ap().rearrange("(c p) d -> p c d", p=128))
            w_sb[name] = t
        ones = consts.tile([128, 128], mm_dt, tag="ones")
        ident = consts.tile([128, 128], mm_dt, tag="ident")
        if mm_dt == BF16:
            import ml_dtypes

            ones_c = nc.inline_tensor(
                np.ones((128, 128), dtype=ml_dtypes.bfloat16), name="ones_c"
            )
            ident_c = nc.inline_tensor(
                np.eye(128, dtype=np.float32).astype(ml_dtypes.bfloat16),
                name="ident_c",
            )
            nc.gpsimd.dma_start(out=ones, in_=ones_c.ap())
            nc.gpsimd.dma_start(out=ident, in_=ident_c.ap())
        else:
            ones_f32 = consts.tile([128, 128], F32, tag="ones_f32")
            nc.vector.memset(ones_f32, 1.0)
            nc.vector.tensor_copy(ones, ones_f32)
            ident_f32 = consts.tile([128, 128], F32, tag="ident_f32")
            make_identity(nc, ident_f32)
            nc.vector.tensor_copy(ident, ident_f32)

        kt_sb = persist.tile([128, S_kv], mm_dt, tag="kt")  # KT [D, S_kv]
        v_sb = persist.tile([128, NKC, D_], mm_dt, tag="v")  # V chunks [k128, D]
        qt_sb = persist.tile([128, S_q], mm_dt, tag="qt")  # QT [D, S_q]

        xT_r = xT.ap().rearrange("(c p) s -> p c s", p=128)

        def proj_block(kb, ps_proj, ps_tr, tr_tag="ps_tr"):
            """Emit projection work for k-block kb as a list of closures so the
            caller can interleave attention iterations between the parts."""
            xblk = xpool.tile([128, EC, KB], mm_dt, tag="xblk", name=f"xblk_{kb}")
            cpd = max(EC // 4, 1) if kb else 1 if EC < 4 else 2
            for di in range(0, EC, cpd):
                eng = nc.scalar if (kb == 0 and (di // cpd) % 2) else nc.sync
                eng.dma_start(
                    out=xblk[:, di : di + cpd, :],
                    in_=xT_r[:, di : di + cpd, kb * KB : (kb + 1) * KB],
                )

            def part_kt():
                ps_kt = ps_proj.tile([128, KB], F32, tag="ps_proj", name=f"ps_kt_{kb}")
                for c in range(EC):
                    nc.tensor.matmul(
                        ps_kt,
                        lhsT=w_sb["wk"][:, c, :],
                        rhs=xblk[:, c, :],
                        start=(c == 0),
                        stop=(c == EC - 1),
                    )
                nc.vector.tensor_copy(kt_sb[:, kb * KB : (kb + 1) * KB], ps_kt)

            def part_vt():
                ps_vt = ps_proj.tile([128, KB], F32, tag="ps_proj", name=f"ps_vt_{kb}")
                for c in range(EC):
                    nc.tensor.matmul(
                        ps_vt,
                        lhsT=w_sb["wv"][:, c, :],
                        rhs=xblk[:, c, :],
                        start=(c == 0),
                        stop=(c == EC - 1),
                    )
                vt_tmp = vt_pool.tile([128, KB], mm_dt, tag="vt_tmp", name=f"vt_{kb}")
                nc.vector.tensor_copy(vt_tmp, ps_vt)
                for t_ in range(TPB):
                    kc = kb * TPB + t_
                    ps_v = ps_tr.tile([128, 128], mm_dt, tag=tr_tag, name=f"ps_v_{kc}")
                    nc.tensor.transpose(
                        ps_v, vt_tmp[:, t_ * 128 : (t_ + 1) * 128], ident
                    )
                    # phase-1b shares the kernel with exp-saturated ACT: keep
                    # its V-tile evacuations off the scalar engine.
                    if kb < NKB // 2:
                        nc.scalar.copy(v_sb[:, kc, :], ps_v)
                    else:
                        nc.vector.tensor_copy(v_sb[:, kc, :], ps_v)

            def part_qt():
                if kb * KB >= S_q:
                    return
                qw = min(KB, S_q - kb * KB)
                ps_qt = ps_proj.tile([128, KB], F32, tag="ps_proj", name=f"ps_qt_{kb}")
                for c in range(EC):
                    nc.tensor.matmul(
                        ps_qt[:, :qw],
                        lhsT=w_sb["wq"][:, c, :],
                        rhs=xblk[:, c, :qw],
                        start=(c == 0),
                        stop=(c == EC - 1),
                    )
                nc.vector.tensor_copy(qt_sb[:, kb * KB : kb * KB + qw], ps_qt[:, :qw])

            return [part_kt, part_vt, part_qt]

        class AttnEmitter:
            """Attention over a fixed qb set. Iterations (kp, qb) are fed in
            order; scores are emitted 1 ahead, denominator ones-matmuls are
            delayed 2 iterations behind their DVE pair-sum trees."""

            def __init__(self, qbs, ps_s_pool, ps_od, dtmp_pool=None):
                self.qbs = qbs
                self.ps_s_pool = ps_s_pool
                self.ps_od = ps_od
                self.dtmp_pool = dtmp_pool
                self.ps_o = {}
                self.ps_d = {}
                self.d_acc = {}
                for qb in qbs:
                    self.ps_o[qb] = ps_od.tile(
                        [128, QB], F32, tag="ps_od", name=f"ps_o_{qb}"
                    )
                    if dtmp_pool is None:
                        self.ps_d[qb] = ps_od.tile(
                            [128, QB], F32, tag="ps_od", name=f"ps_d_{qb}"
                        )
                    else:
                        self.d_acc[qb] = persist.tile(
                            [128, QB], F32, tag=f"d_acc_{qb}", name=f"d_acc_{qb}"
                        )
                self.held = {qb: [None, None] for qb in qbs}
                self.ngroups = NKP // 4 + (1 if NKP % 4 else 0)
                self.gidx = {qb: 0 for qb in qbs}
                self.denom_q = []
                self.idx = 0
                self.pending = None
                self.pending_it = None

            def _scores(self, it):
                kp, qb = it
                ps_s = self.ps_s_pool.tile(
                    [128, 2, QB], F32, tag="ps_s", name=f"ps_s_{kp}_{qb}"
                )
                for j in range(2):
                    kc = 2 * kp + j
                    nc.tensor.matmul(
                        ps_s[:, j, :],
                        lhsT=kt_sb[:, kc * 128 : (kc + 1) * 128],
                        rhs=qt_sb[:, qb * QB : (qb + 1) * QB],
                        start=True,
                        stop=True,
                    )
                return ps_s

            def _emit_denoms(self, before_idx):
                while self.denom_q and self.denom_q[0][0] <= before_idx:
                    _, dqb, g, quad = self.denom_q.pop(0)
                    if self.dtmp_pool is None:
                        nc.tensor.matmul(
                            self.ps_d[dqb],
                            lhsT=ones,
                            rhs=quad,
                            start=(g == 0),
                            stop=(g == self.ngroups - 1),
                        )
                    else:
                        dtmp = self.dtmp_pool.tile(
                            [128, QB], F32, tag="dtmp", name=f"dtmp_{dqb}_{g}"
                        )
                        nc.tensor.matmul(
                            dtmp, lhsT=ones, rhs=quad, start=True, stop=True
                        )
                        if g == 0:
                            nc.vector.tensor_copy(self.d_acc[dqb], dtmp)
                        else:
                            nc.vector.tensor_add(
                                self.d_acc[dqb], self.d_acc[dqb], dtmp
                            )

            def step(self, it, next_it):
                if self.pending is None:
                    self.pending = self._scores(it)
                    self.pending_it = it
                assert self.pending_it == it
                ps_s = self.pending
                if next_it is not None:
                    self.pending = self._scores(next_it)
                    self.pending_it = next_it
                else:
                    self.pending = None
                self._emit_denoms(self.idx - 3)
                kp, qb = it
                p_sb = p_pool.tile(
                    [128, 2, QB], mm_dt, tag="p_sb", name=f"p_sb_{kp}_{qb}"
                )
                nc.scalar.activation(p_sb, ps_s, mybir.ActivationFunctionType.Exp)
                for j in range(2):
                    kc = 2 * kp + j
                    nc.tensor.matmul(
                        self.ps_o[qb],
                        lhsT=v_sb[:, kc, :],
                        rhs=p_sb[:, j, :],
                        start=(kp == 0 and j == 0),
                        stop=(kp == NKP - 1 and j == 1),
                    )
                pair = pair_pool.tile(
                    [128, QB], mm_dt, tag="pair", name=f"pair_{kp}_{qb}"
                )
                nc.vector.tensor_add(pair, p_sb[:, 0, :], p_sb[:, 1, :])
                lvl = self.held[qb]
                cur = pair
                placed = False
                for li in range(len(lvl)):
                    if lvl[li] is None:
                        lvl[li] = cur
                        placed = True
                        break
                    nxt = pair_pool.tile(
                        [128, QB], mm_dt, tag="pair", name=f"red{li}_{kp}_{qb}"
                    )
                    nc.vector.tensor_add(nxt, lvl[li], cur)
                    lvl[li] = None
                    cur = nxt
                if not placed:
                    self.denom_q.append((self.idx, qb, self.gidx[qb], cur))
                    self.gidx[qb] += 1
                self.idx += 1

            def finish(self):
                for qb in self.qbs:
                    lvl = self.held[qb]
                    cur = None
                    for li in range(len(lvl)):
                        if lvl[li] is not None:
                            if cur is None:
                                cur = lvl[li]
                            else:
                                nxt = pair_pool.tile(
                                    [128, QB], mm_dt, tag="pair", name=f"fl{li}_{qb}"
                                )
                                nc.vector.tensor_add(nxt, lvl[li], cur)
                                cur = nxt
                            lvl[li] = None
                    if cur is not None:
                        self.denom_q.append((self.idx, qb, self.gidx[qb], cur))
                        self.gidx[qb] += 1
                self._emit_denoms(self.idx)
                for qb in self.qbs:
                    d_src = (
                        self.ps_d[qb] if self.dtmp_pool is None else self.d_acc[qb]
                    )
                    rec = o_pool.tile([128, QB], F32, tag="rec")
                    nc.vector.reciprocal_approx_fast(out=rec, in_=d_src)
                    o_sb = o_pool.tile([128, QB], F32, tag="o_sb")
                    nc.vector.tensor_mul(o_sb, self.ps_o[qb], rec)
                    nc.sync.dma_start(
                        out=oT.ap()[:, qb * QB : (qb + 1) * QB],
                        in_=o_sb,
                    )

        def run_iters(att, its):
            for i, it in enumerate(its):
                att.step(it, its[i + 1] if i + 1 < len(its) else None)

        NKB1 = NKB // 2  # phase-1a blocks
        can_fuse = (
            fused and NKB >= 2 and NQB >= 2 and NQB % 2 == 0
            and (NQB // 2) * QB <= NKB1 * KB  # qbsA QT done in phase 1a
        )

        if can_fuse and fused == "flat":
            # Flat single-scope PSUM layout: o-accumulators (2 banks) recycle
            # across the two qb generations, all denominators accumulate in
            # SBUF, proj+transpose share one slot, scores double-buffered.
            # Attention starts at kb2 with a strict one-block availability lag
            # (reads must be emitted after their producer writes: Tile gives
            # stale data, not a dependency, for read-before-write emission).
            qbsA = tuple(range(NQB // 2))
            qbsB = tuple(range(NQB // 2, NQB))
            itersA = [(kp, qb) for kp in range(NKP) for qb in qbsA]
            with tc.tile_pool(name="ps_o", bufs=2, space="PSUM") as ps_o_pool, \
                 tc.tile_pool(name="ps_sF", bufs=2, space="PSUM") as ps_sF, \
                 tc.tile_pool(name="ps_pt", bufs=1, space="PSUM") as ps_pt, \
                 tc.tile_pool(name="ps_dt", bufs=1, space="PSUM") as ps_dt:
                attA = AttnEmitter(qbsA, ps_sF, ps_o_pool, dtmp_pool=ps_dt)
                pos = 0
                for kb in range(NKB):
                    parts = proj_block(kb, ps_pt, ps_pt, tr_tag="ps_proj")
                    if kb == NKB - 1:
                        quota = len(itersA) - pos
                    elif kb < 2:
                        quota = 0
                    else:
                        safe = len(qbsA) * ((TPB * kb) // 2) - 1
                        quota = max(min(safe, len(itersA)) - pos, 0)
                    take = itersA[pos : pos + quota]
                    pos += quota
                    nparts = len(parts)
                    per = (len(take) + nparts) // (nparts + 1)
                    ti = 0
                    for part in parts:
                        part()
                        for it in take[ti : ti + per]:
                            gi = itersA.index(it)
                            nxt = itersA[gi + 1] if gi + 1 < len(itersA) else None
                            attA.step(it, nxt)
                        ti += per
                    for it in take[ti:]:
                        gi = itersA.index(it)
                        nxt = itersA[gi + 1] if gi + 1 < len(itersA) else None
                        attA.step(it, nxt)
                attA.finish()
                for qi in range(0, len(qbsB), 2):
                    qbs = qbsB[qi : qi + 2]
                    attB = AttnEmitter(qbs, ps_sF, ps_o_pool, dtmp_pool=ps_dt)
                    run_iters(attB, [(kp, qb) for kp in range(NKP) for qb in qbs])
                    attB.finish()
        elif not can_fuse:
            # ---- unfused: projections, then attention per qb pair ----
            with tc.tile_pool(name="ps_proj", bufs=3, space="PSUM") as ps_proj, \
                 tc.tile_pool(name="ps_tr", bufs=2, space="PSUM") as ps_tr:
                for kb in range(NKB):
                    for part in proj_block(kb, ps_proj, ps_tr):
                        part()
            with tc.tile_pool(name="ps_s", bufs=2, space="PSUM") as ps_s_pool, \
                 tc.tile_pool(name="ps_od", bufs=4, space="PSUM") as ps_od:
                for qp in range(NQB // 2):
                    qbs = (2 * qp, 2 * qp + 1)
                    att = AttnEmitter(qbs, ps_s_pool, ps_od)
                    run_iters(
                        att, [(kp, qb) for kp in range(NKP) for qb in qbs]
                    )
                    att.finish()
        else:
            qbsA = tuple(range(NQB // 2))
            qbsB = tuple(range(NQB // 2, NQB))
            # phase 1a: first half of the projection blocks
            with tc.tile_pool(name="ps_proj", bufs=3, space="PSUM") as ps_proj, \
                 tc.tile_pool(name="ps_tr", bufs=2, space="PSUM") as ps_tr:
                for kb in range(NKB1):
                    for part in proj_block(kb, ps_proj, ps_tr):
                        part()
            # phase 1b: remaining projection blocks with qbsA attention
            # interleaved between the projection parts.
            itersA = [(kp, qb) for kp in range(NKP) for qb in qbsA]
            with tc.tile_pool(name="ps_proj2", bufs=1, space="PSUM") as ps_proj2, \
                 tc.tile_pool(name="ps_s1", bufs=2, space="PSUM") as ps_s1, \
                 tc.tile_pool(name="ps_odA", bufs=2, space="PSUM") as ps_odA, \
                 tc.tile_pool(name="ps_dtmp", bufs=1, space="PSUM") as ps_dtmp:
                attA = AttnEmitter(qbsA, ps_s1, ps_odA, dtmp_pool=ps_dtmp)
                nkb2 = NKB - NKB1
                pos = 0
                for i, kb in enumerate(range(NKB1, NKB)):
                    parts = proj_block(kb, ps_proj2, ps_proj2, tr_tag="ps_proj")
                    avail_kp = ((kb + 1) * TPB) // 2
                    if i == nkb2 - 1:
                        quota = len(itersA) - pos
                    else:
                        quota = min(
                            (len(itersA) * (i + 1)) // nkb2 - pos,
                            len(qbsA) * avail_kp - pos,
                        )
                    take = itersA[pos : pos + quota]
                    pos += quota
                    nparts = len(parts)
                    per = (len(take) + nparts) // (nparts + 1)
                    ti = 0
                    for part in parts:
                        part()
                        sub = take[ti : ti + per]
                        for it in sub:
                            gi = itersA.index(it)
                            nxt = itersA[gi + 1] if gi + 1 < len(itersA) else None
                            attA.step(it, nxt)
                        ti += len(sub)
                    for it in take[ti:]:
                        gi = itersA.index(it)
                        nxt = itersA[gi + 1] if gi + 1 < len(itersA) else None
                        attA.step(it, nxt)
                attA.finish()
            # phase 2: qbsB attention
            with tc.tile_pool(name="ps_s", bufs=2, space="PSUM") as ps_s_pool, \
                 tc.tile_pool(name="ps_od", bufs=4, space="PSUM") as ps_od:
                for qi in range(0, len(qbsB), 2):
                    qbs = qbsB[qi : qi + 2]
                    att = AttnEmitter(qbs, ps_s_pool, ps_od)
                    run_iters(
                        att, [(kp, qb) for kp in range(NKP) for qb in qbs]
                    )
                    att.finish()

    nc.compile()
    return nc


_NC_CACHE = {}


def _get_nc(key, *args, **kwargs):
    if key not in _NC_CACHE:
        _NC_CACHE[key] = build_nc(*args, **kwargs)
    return _NC_CACHE[key]


def run_cores(nc, in_maps, **kwargs):
    core_ids = list(range(len(in_maps)))
    return run_bass_kernel_spmd(nc, in_maps, core_ids=core_ids, **kwargs)


def run_cores_profiled(nc, in_maps, trace_cores=(0,)):
    """Run via PJRT with NRT profiling (the antenv hook is missing in this
    container, so drive the ctypes profile start/stop directly)."""
    import glob
    import tempfile

    import gauge.profiler
    from concourse import bass2jax
    from concourse._compat import FishPath
    from trn_agent_boot.trn_boot import _ntff_profile_via_ctypes

    hook = _ntff_profile_via_ctypes("/opt/axon/libaxon_pjrt.so")
    neff_dir = tempfile.mkdtemp(prefix="attn_prof_")
    with hook(neff_dir, list(trace_cores)):
        results = bass2jax.run_bass_via_pjrt(nc, in_maps, n_cores=len(in_maps))
    ntffs = glob.glob(neff_dir + "/*_body*.ntff")
    if not ntffs:
        print("WARNING: no NTFFs captured in", neff_dir)
        return results, None, None
    profile = gauge.profiler.Profile(
        profile_path=FishPath(neff_dir),
        kernel_dev_mode=True,
        profile_on_exit=False,
        bass_kernel=nc.m,
        offline_processing=True,
        fname="*_body*",
        metadata={"artifacts_path": neff_dir},
    )
    prs = profile.to_perfetto(model_index=tuple(trace_cores))
    exec_ns = max(pr.exec_time_ns for pr in prs)
    return results, exec_ns, prs


VARIANT = "bf16"  # "f32r" or "bf16"


def _cvt(a, variant):
    if variant == "bf16":
        import ml_dtypes

        return np.ascontiguousarray(a).astype(ml_dtypes.bfloat16)
    return round_fp32r(a)


def kernel(x, Wq, Wk, Wv, _trace=False, _trace_cores=(0,), _variant=None):
    variant = _variant or VARIANT
    x = np.asarray(x, dtype=np.float32)
    scale = 1.0 / math.sqrt(Wq.shape[1])
    wq_s = _cvt(np.asarray(Wq, np.float32) * scale, variant)
    wk_ = _cvt(np.asarray(Wk, np.float32), variant)
    wv_ = _cvt(np.asarray(Wv, np.float32), variant)

    mm_dt = BF16 if variant == "bf16" else F32R
    nc = _get_nc(
        "full_" + variant, S, QH, E, D, mm_dt=mm_dt,
        fused=(variant == "bf16"),
    )
    in_maps = []
    for c in range(N_CORES):
        b, h = divmod(c, 2)
        xb = x[b]
        if h == 0:
            xr = xb
        else:
            xr = np.concatenate([xb[QH:], xb[:QH]], axis=0)
        in_maps.append(
            {
                "xT": _cvt(xr.T, variant),
                "wq": wq_s,
                "wk": wk_,
                "wv": wv_,
            }
        )
    if _trace:
        results, exec_ns, prs = run_cores_profiled(nc, in_maps, trace_cores=_trace_cores)
        kernel.last_exec_time_ns = exec_ns
        kernel.last_prs = prs
    else:
        results = run_cores(nc, in_maps).results
    out = np.empty((B, S, D), dtype=np.float32)
    for c in range(N_CORES):
        b, h = divmod(c, 2)
        out[b, h * QH : (h + 1) * QH, :] = results[c]["oT"].T
    return out
